# revision 1
# baseline (speedup 1.0000x reference)
"""BrightnessLoss Trainium2 kernel (raw Bass, 8-core data parallel).

reference:
    V(x)   = max_c(clip(x, 0, 1))        over channel dim (RGB)
    result = mean(|V(pred) - V(target)|) over (N, H, W)

Identities used on device:
    clip(max(r,g,b),0,1) == max_c(clip(x,0,1))          (clip is monotone)
    W := relu(1 - relu(m)) == 1 - clip(m, 0, 1)
    |Vp - Vt| == |Wp - Wt|
    sum|Wp - Wt| == 2*sum max(Wp,Wt) - sum(Wp + Wt)

The stream is the roofline: ~25.2 MB of fp32 input per core, and the 16
SDMA engines cap at ~24 GB/s each with 4 KB packets (per-packet
overhead), a bit more with 8 KB packets. So the design goal is a gapless
two-ring DMA stream of the largest-possible contiguous runs, with compute
strictly faster than arrival:

  - DMA "groups" cover column ranges of each image; even groups ride the
    Sync HWDGE ring, odd groups the ACT HWDGE ring, each carrying pred
    then targ back-to-back (12.6 MB per ring). Image 0 leads with paired
    small groups (128/128/256/256/640/640 cols) so compute starts ~1 us
    into the stream with both rings' packet sizes matched; image 3
    trails with the mirror (640/640/256/256/128/128) so the closing
    dependency chain is short.
  - 6 group slots [P, 2, 3, w] (both sides side-by-side) keep each ring
    ~3 transfers deep, so the rings never run dry.
  - Compute "units" (<=1024 cols) subdivide groups. Per unit, both sides
    in one wide op:
        DVE TT   m = max(R2, G2)          [P, 2, w]
        DVE STT  u = max(max(m,0), B2)    [P, 2, w]
        ACT      W = Relu(1 - u) (bf16),  accum_out = sum(Wp)+sum(Wt)
        DVE STT  max(Wp, Wt) (bf16),      accum_out = sum
    DVE needs ~5.8 us per 1024-col unit vs ~7.3 us arrival, so it stays
    caught up and the tail after the last packet is just the last small
    unit's chain. Partials go out in two DMAs (bulk early, last units at
    the end). Host combines in float64.
"""

import numpy as np

N_CORES = 8
N_IMG = 4  # 32 / 8
C = 3
P = 128
F = 2048  # 512*512 / 128
N_PIX = 32 * 512 * 512
FC = 1024  # max compute-unit width
S_G = 6  # group slot depth: deep enough that ring issues wait only on
# ancient compute (STT of g-6), keeping both rings' queues full from t=0
# Small transfers are expensive two ways: sub-4KB DRAM runs drop the
# per-SDMA-engine rate, and each transfer carries a ~1.5us ring bubble.
# So the head/tail use the FEWEST small pieces that still let compute
# start early / finish promptly: one 256 + one 768 per edge, staggered
# across the two rings, uniform 1024s everywhere else.
HEAD_SPLIT = (256, 768, 1024)  # image 0 groups (sum = F)
TAIL_SPLIT = (1024, 768, 256)  # last image groups (sum = F)


def _plan():
    """groups: (img, col_off, width); units: (grp_idx, off_in_grp, width).
    One group = one DMA transfer per side = one compute unit (<= FC cols):
    4 KB DRAM runs already saturate the per-SDMA-engine rate, and unit-
    sized transfers keep the slot-WAR release chain fine-grained."""
    groups = []
    for img in range(N_IMG):
        if img == 0:
            widths = HEAD_SPLIT
        elif img == N_IMG - 1:
            widths = TAIL_SPLIT
        else:
            widths = (FC,) * (F // FC)
        o = 0
        for w in widths:
            groups.append((img, o, w))
            o += w
        assert o == F
    units = []
    for g, (_img, _off, w) in enumerate(groups):
        o = 0
        while o < w:
            uw = min(FC, w - o)
            units.append((g, o, uw))
            o += uw
    return groups, units


def _build_program():
    from contextlib import ExitStack

    import concourse.bass as bass
    import concourse.mybir as mybir

    fp32 = mybir.dt.float32
    bf16 = mybir.dt.bfloat16
    Alu = mybir.AluOpType
    Act = mybir.ActivationFunctionType

    groups, units = _plan()
    n_groups = len(groups)
    n_units = len(units)
    last_unit_of = {}
    for u, (g, _o, _w) in enumerate(units):
        last_unit_of[g] = u
    slot_w = [
        max(groups[g][2] for g in range(s, n_groups, S_G)) for s in range(S_G)
    ]

    # detect_race_conditions=False: the raw-mode CoreSim race detector can't
    # see same-engine program-order (DVE m1 -> STT RAW); hardware engines
    # execute in order.
    # The construction-time all_engine_barrier orders the const-tile memsets
    # against engines that read them; this kernel uses only instruction
    # immediates, so skip it and let the engines reach first work sooner.
    _orig_barrier = bass.Bass.all_engine_barrier
    bass.Bass.all_engine_barrier = lambda *a, **k: None
    try:
        nc = bass.Bass(
            "TRN2",
            target_bir_lowering=False,
            debug=False,
            detect_race_conditions=False,
        )
    finally:
        bass.Bass.all_engine_barrier = _orig_barrier
    pred = nc.dram_tensor("pred", [N_IMG, C, P, F], fp32, kind="ExternalInput").ap()
    targ = nc.dram_tensor("target", [N_IMG, C, P, F], fp32, kind="ExternalInput").ap()
    out = nc.dram_tensor(
        "partials", [P, 2 * n_units], fp32, kind="ExternalOutput"
    ).ap()

    with ExitStack() as ctx:
        sb = lambda name, shape, dt=fp32: ctx.enter_context(
            nc.sbuf_tensor(name, shape, dt)
        )
        sem = lambda name: ctx.enter_context(nc.semaphore(name))

        # one slot holds BOTH sides of a group: [P, side, chan, slot_w]
        inb = [sb(f"in{s}", [P, 2, C, slot_w[s]]) for s in range(S_G)]
        ub = [sb(f"u{s}", [P, 2 * FC]) for s in range(2)]
        wb = [sb(f"w{s}", [P, 2 * FC], bf16) for s in range(2)]
        m1 = sb("m1", [P, 3 * FC])  # third segment: hoisted pred-side scratch
        scr = sb("stt_scratch", [P, FC], bf16)
        acc = sb("acc", [P, 2 * n_units])

        inp_sem = [sem("inp0"), sem("inp1")]  # pred side, by ring parity
        int_sem = [sem("int0"), sem("int1")]  # targ side, by ring parity
        u_sem = sem("u")  # +1 per unit after DVE STT (inb consumed)
        act_sem = sem("act")  # +1 per unit after ACT (ub consumed, wb+acc ready)
        gp_sem = sem("gp")  # +1 per unit after DVE accum (wb consumed)
        out_sem = sem("outd")

        def dma_in(eng, side_idx, g):
            img, off, w = groups[g]
            side = (pred, targ)[side_idx]
            s_sem = (inp_sem, int_sem)[side_idx]
            src = side[img, :, :, off : off + w].rearrange("c p f -> p c f")
            eng.dma_start(
                out=inb[g % S_G][:, side_idx, :, :w],
                in_=src,
            ).then_inc(s_sem[g % 2], 16)

        block = ctx.enter_context(nc.Block(no_gpsimd_drain=True))

        @block.sync
        def _(sync):
            # even units ride the SP ring (pred+targ back-to-back); odd units
            # are issued from the ACT stream (second HWDGE ring). The two
            # rings stay one unit apart, which keeps their DRAM read streams
            # decorrelated — issuing each side on its own ring measurably
            # tanks the aggregate rate.
            for g in range(0, n_groups, 2):
                if g >= S_G:
                    # WAR inb[g%S_G]: unit g-S_G's STT was its last reader
                    sync.wait_ge(u_sem, g - S_G + 1)
                dma_in(sync, 0, g)
                dma_in(sync, 1, g)
            if n_units > 2:
                # bulk of partials early; only the last 2 units' cols remain.
                # gp_sem >= k implies act_sem >= k (accum u waits ACT u), so
                # both engines' acc columns for units < k are final.
                sync.wait_ge(gp_sem, n_units - 2)
                sync.dma_start(
                    out=out[:, : 2 * (n_units - 2)],
                    in_=acc[:, : 2 * (n_units - 2)],
                ).then_inc(out_sem, 16)
            sync.wait_ge(gp_sem, n_units)
            # No out_sem wait after the final write: the block-exit drain
            # fences the HWDGE ring before NEFF completion.
            sync.dma_start(
                out=out[:, 2 * max(0, n_units - 2) :],
                in_=acc[:, 2 * max(0, n_units - 2) :],
            ).then_inc(out_sem, 16)

        @block.vector
        def _(vector):
            def accum(u):
                # max(Wp, Wt) elementwise (bf16), accum_out = row sum
                w = units[u][2]
                vector.wait_ge(act_sem, u + 1)
                vector.scalar_tensor_tensor(
                    scr[:, :w],
                    wb[u % 2][:, :w],
                    0.0,
                    wb[u % 2][:, w : 2 * w],
                    op0=Alu.bypass,
                    op1=Alu.max,
                    accum_out=acc[:, 2 * u : 2 * u + 1],
                ).then_inc(gp_sem, 1)

            # Units near the stream's edges run per-SIDE (pred ops while
            # targ is still in flight — pred lands half a group earlier), so
            # the residual work after the LAST byte arrives is halved. Mid-
            # stream units keep the fused wide ops (fewer op overheads).
            unfused = {0, n_units - 3, n_units - 2, n_units - 1}

            def side_ops(u, s, m_off=None):
                g, o, w = units[u]
                t = inb[g % S_G]
                mo = s * w if m_off is None else m_off
                vector.tensor_max(
                    m1[:, mo : mo + w],
                    t[:, s, 0, o : o + w],
                    t[:, s, 1, o : o + w],
                )
                st = vector.scalar_tensor_tensor(
                    ub[u % 2][:, s * w : (s + 1) * w],
                    m1[:, mo : mo + w],
                    0.0,
                    t[:, s, 2, o : o + w],
                    op0=Alu.max,
                    op1=Alu.max,
                )
                return st

            # the pred side of unit n-3 is hoisted before unit n-4: its data
            # lands while DVE waits for n-4's targ half, pulling ~2.4us of
            # work out of the post-stream critical tail. It uses the third
            # m1 segment so n-4's fused scratch isn't clobbered.
            hoist = n_units - 3

            for u in range(n_units):
                g, o, w = units[u]
                t = inb[g % S_G]
                if u == n_units - 4 and hoist in unfused:
                    gh = units[hoist][0]
                    vector.wait_ge(inp_sem[gh % 2], 16 * (gh // 2 + 1))
                    # WAR on ub[hoist%2]: ACT of unit hoist-2 (its reader)
                    vector.wait_ge(act_sem, hoist - 1)
                    side_ops(hoist, 0, m_off=2 * FC)
                if u in unfused:
                    if u == hoist:
                        # pred side already hoisted; targ side only
                        vector.wait_ge(int_sem[g % 2], 16 * (g // 2 + 1))
                        side_ops(u, 1).then_inc(u_sem, 1)
                        accum(u - 1)
                        continue
                    vector.wait_ge(inp_sem[g % 2], 16 * (g // 2 + 1))
                    if u >= 2:
                        # WAR on ub[u%2]: ACT's W of unit u-2 (its reader)
                        vector.wait_ge(act_sem, u - 1)
                    side_ops(u, 0)
                    vector.wait_ge(int_sem[g % 2], 16 * (g // 2 + 1))
                    side_ops(u, 1).then_inc(u_sem, 1)
                    if u > 0:
                        # accum AFTER both sides: its act_sem wait must not
                        # gate the targ-side ops (that serializes the tail)
                        accum(u - 1)
                else:
                    vector.wait_ge(inp_sem[g % 2], 16 * (g // 2 + 1))
                    vector.wait_ge(int_sem[g % 2], 16 * (g // 2 + 1))
                    mv = m1[:, : 2 * w].rearrange("p (s w) -> p s w", s=2)
                    uv = ub[u % 2][:, : 2 * w].rearrange("p (s w) -> p s w", s=2)
                    vector.tensor_max(
                        mv, t[:, :, 0, o : o + w], t[:, :, 1, o : o + w]
                    )
                    if u >= 2:
                        vector.wait_ge(act_sem, u - 1)
                    vector.scalar_tensor_tensor(
                        uv,
                        mv,
                        0.0,
                        t[:, :, 2, o : o + w],
                        op0=Alu.max,
                        op1=Alu.max,
                    ).then_inc(u_sem, 1)
                    accum(u - 1)
            accum(n_units - 1)

        @block.scalar
        def _(scalar):
            # odd units' input DMAs ride the ACT HWDGE ring. Units 1 and 3 go
            # up front (fresh slots, no WAR); unit n+S_G is placed right
            # after ACT(n), whose u_sem wait (>= n+1) covers the WAR for slot
            # (n+S_G) % S_G (last STT reader was unit n).
            for g in range(1, min(S_G, n_groups), 2):
                dma_in(scalar, 0, g)
                dma_in(scalar, 1, g)
            for n in range(n_units):
                w = units[n][2]
                scalar.wait_ge(u_sem, n + 1)
                if n >= 2:
                    # WAR on wb[n%2]: accum of unit n-2 (its reader)
                    scalar.wait_ge(gp_sem, n - 1)
                scalar.activation(
                    wb[n % 2][:, : 2 * w],
                    ub[n % 2][:, : 2 * w],
                    Act.Relu,
                    bias=1.0,
                    scale=-1.0,
                    accum_out=acc[:, 2 * n + 1 : 2 * n + 2],
                ).then_inc(act_sem, 1)
                if n + S_G < n_groups and (n + S_G) % 2 == 1:
                    dma_in(scalar, 0, n + S_G)
                    dma_in(scalar, 1, n + S_G)

        # Skip the Block-exit all-engine barrier (~4.3us): every cross-engine
        # dependency is semaphore-gated and the per-engine exit drains
        # (no_gpsimd_drain path) still fence the DMA rings, so engines may
        # halt independently — NEFF completion waits for all engines anyway.
        nc.all_engine_barrier = lambda *a, **k: None

    del nc.all_engine_barrier  # restore class method
    return nc


_program = None


def _get_program():
    global _program
    if _program is None:
        _program = _build_program()
    return _program


def _finish(partials_list):
    """partials_list: per-core [P, 2*n_units] f32 with cols per unit:
    [sum max(Wp,Wt), sum Wp + sum Wt].
    sum|Vp-Vt| = 2*sum(max) - (sum Wp + sum Wt)."""
    total = np.float64(0.0)
    for p in partials_list:
        p = p.astype(np.float64)
        total += 2.0 * p[:, 0::2].sum() - p[:, 1::2].sum()
    return np.array(total / N_PIX, dtype=np.float32)


def kernel(pred: np.ndarray, target: np.ndarray) -> np.ndarray:
    from concourse.bass_utils import run_bass_kernel_spmd

    nc = _get_program()
    pred = np.ascontiguousarray(pred, dtype=np.float32).reshape(
        N_CORES, N_IMG, C, P, F
    )
    target = np.ascontiguousarray(target, dtype=np.float32).reshape(
        N_CORES, N_IMG, C, P, F
    )
    in_maps = [{"pred": pred[i], "target": target[i]} for i in range(N_CORES)]
    res = run_bass_kernel_spmd(nc, in_maps, list(range(N_CORES)))
    return _finish([r["partials"] for r in res.results])



# revision 2
# speedup vs baseline: 1.0517x; 1.0517x over previous
"""BrightnessLoss Trainium2 kernel (raw Bass, 8-core data parallel) — final (v8).

reference:
    V(x)   = max_c(clip(x, 0, 1))        over channel dim (RGB)
    result = mean(|V(pred) - V(target)|) over (N, H, W)

Pipeline (HW-measured; see bench1.py, probes, v2-v5 traces):
  - Bulk stream: ONE SWDGE queue (gpsimd dma_start) casting fp32->bf16
    in flight (~420 GB/s read-side, equal to dual HWDGE); bf16 halves
    DVE marginal cost (TT max 2x, TS clip 4x).
  - SWDGE's descriptor rings contend with SDMA engine 15's AXI port:
    under full queue load that one engine runs 10-20% slow and its equal
    packet share finishes 0-11 us after the rest (v2/v5 bimodality;
    pure-HWDGE traces show zero skew). Fix, validated by probe: a DMA
    whose dest covers partitions 0:120 is dealt to engines 0-14 ONLY
    (outer-dim divisor rule, 8 partitions each - same per-engine rate),
    so every SWDGE transfer here writes partitions 0:120 and engine 15
    carries no stream data at all. The remaining partitions 120:128
    (6.25% of bytes) ride the two idle HWDGE rings as fp32 (sync=pred,
    scalar=targ; 12 transfers/side of [32,512] due to the 3-dim AP
    limit), repacked across all 128 partitions and processed as one
    extra fp32 DVE unit mid-order.
  - Partition-sliced transfers may increment their semaphore only
    15x (one per engine), so each group gets its own semaphore waited
    at >=30 - correct whether a transfer contributes 15 or 16.
  - Per unit (both sides fused): DVE TT max(R,G); TT max(.,B); TS clip01
    (accum_out on TS is a 3.3x slow path - keep it off); TT sub -> d.
    ACT: Abs(d) accum_out -> acc[:, u]. Final (256-col) unit does
    |d| = max(-d, d) + accum on DVE instead, skipping the last
    cross-engine hop. Host sums partials in f64.
"""

import numpy as np

N_CORES = 8
N_IMG = 4  # 32 / 8
C = 3
P = 128
PS = 120  # partitions streamed via SWDGE (engines 0-14)
F = 2048  # 512*512 / 128
N_PIX = 32 * 512 * 512
S_G = 5  # slot ring depth

# SWDGE groups, issue order == DVE order (es unit spliced in at ES_POS)
GROUPS = (
    (0, 0, 1024),
    (0, 1024, 1024),
    (1, 0, 2048),
    (2, 0, 2048),
    (3, 0, 1024),
    (3, 1024, 512),
    (3, 1536, 256),
    (3, 1792, 256),
)
N_G = len(GROUPS)
ES_POS = 3  # DVE position of the partitions-120:128 fp32 unit
N_U = N_G + 1
W_MAX = 2048
ES_W = 512  # es unit free width: 8 partitions x 4 imgs x 4 fh -> 128 x 512


def _build_program():
    from contextlib import ExitStack

    import concourse.bass as bass
    import concourse.mybir as mybir

    fp32 = mybir.dt.float32
    bf16 = mybir.dt.bfloat16
    Alu = mybir.AluOpType
    Act = mybir.ActivationFunctionType

    _orig_barrier = bass.Bass.all_engine_barrier
    bass.Bass.all_engine_barrier = lambda *a, **k: None
    try:
        nc = bass.Bass(
            "TRN2",
            target_bir_lowering=False,
            debug=False,
            detect_race_conditions=False,
        )
    finally:
        bass.Bass.all_engine_barrier = _orig_barrier

    pred = nc.dram_tensor("pred", [N_IMG, C, P, F], fp32, kind="ExternalInput").ap()
    targ = nc.dram_tensor("target", [N_IMG, C, P, F], fp32, kind="ExternalInput").ap()
    out = nc.dram_tensor("partials", [P, N_U], fp32, kind="ExternalOutput").ap()

    # DVE-order unit list: ("sw", group_idx) / ("es", None)
    units = [("sw", g) for g in range(N_G)]
    units.insert(ES_POS, ("es", None))

    with ExitStack() as ctx:
        sb = lambda name, shape, dt: ctx.enter_context(
            nc.sbuf_tensor(name, shape, dt)
        )
        sem = lambda name: ctx.enter_context(nc.semaphore(name))

        inb = [sb(f"in{s}", [P, 2, C, W_MAX], bf16) for s in range(S_G)]
        es = sb("es", [P, 2, C, ES_W], fp32)
        ma = sb("ma", [P, 2, W_MAX], bf16)
        mb = sb("mb", [P, 2, W_MAX], bf16)
        vv = sb("vv", [P, 2, W_MAX], bf16)
        maf = sb("maf", [P, 2, ES_W], fp32)
        mbf = sb("mbf", [P, 2, ES_W], fp32)
        vvf = sb("vvf", [P, 2, ES_W], fp32)
        dd = [sb(f"d{i}", [P, W_MAX], bf16) for i in range(3)]
        ddf = sb("ddf", [P, ES_W], fp32)
        absb = sb("absb", [P, W_MAX], bf16)
        acc = sb("acc", [P, N_U], fp32)

        g_sem = [sem(f"g{g}") for g in range(N_G)]  # >=30 <=> both sides landed
        in_p = sem("inp")  # +16 per HWDGE pred slice (sync ring), 12 total
        in_t = sem("int")  # +16 per HWDGE targ slice (scalar ring), 12 total
        swu_sem = sem("swu")  # +1 per SW unit after TT2 (inb slot consumed)
        d_sem = sem("d")  # +1 per unit after sub (d ready for ACT)
        act_sem = sem("act")  # +1 per unit after its |d| accum lands in acc
        out_sem = sem("outd")

        block = ctx.enter_context(nc.Block(no_gpsimd_drain=True))

        @block.gpsimd
        def _(g):
            for s, (img, off, w) in enumerate(GROUPS):
                if s >= S_G:
                    g.wait_ge(swu_sem, s - S_G + 1)
                for side_idx, side in enumerate((pred, targ)):
                    g.dma_start(
                        out=inb[s % S_G][0:PS, side_idx, :, :w],
                        in_=side[img, :, 0:PS, off : off + w].rearrange(
                            "c p f -> p c f"
                        ),
                    ).then_inc(g_sem[s], 16)

        def es_slices(side):
            # partitions 120:128 of every (img, chan): [32, 512] repack;
            # dest partition = img*32 + (p-120)*4 + fh, fh = f // 512
            for img in range(N_IMG):
                for c in range(C):
                    yield (
                        es[img * 32 : (img + 1) * 32, side, c, :],
                        (pred, targ)[side][img, c, PS:P, :].rearrange(
                            "p (fh fl) -> (p fh) fl", fh=4
                        ),
                    )

        @block.sync
        def _(sy):
            # issue es slices only after the stream ramp (group 0 landed):
            # 24 tiny transfers during the ramp halve early SWDGE rate
            sy.wait_ge(g_sem[0], 30)
            for dst, src in es_slices(0):
                sy.dma_start(out=dst, in_=src).then_inc(in_p, 16)
            sy.wait_ge(act_sem, N_U)
            sy.dma_start(out=out, in_=acc[:, :]).then_inc(out_sem, 16)

        @block.vector
        def _(v):
            for u, (kind, gi) in enumerate(units):
                if kind == "sw":
                    img, off, w = GROUPS[gi]
                    t = inb[gi % S_G]
                    v.wait_ge(g_sem[gi], 30)
                    v.tensor_tensor(
                        ma[0:PS, :, :w],
                        t[0:PS, :, 0, :w],
                        t[0:PS, :, 1, :w],
                        op=Alu.max,
                    )
                    v.tensor_tensor(
                        mb[0:PS, :, :w],
                        ma[0:PS, :, :w],
                        t[0:PS, :, 2, :w],
                        op=Alu.max,
                    ).then_inc(swu_sem, 1)
                    v.tensor_scalar(
                        vv[0:PS, :, :w],
                        mb[0:PS, :, :w],
                        0.0,
                        1.0,
                        op0=Alu.max,
                        op1=Alu.min,
                    )
                    if u >= 3:
                        # WAR on dd[u % 3]: ACT's Abs of unit u-3 (its reader)
                        v.wait_ge(act_sem, u - 2)
                    sub = v.tensor_tensor(
                        dd[u % 3][0:PS, :w],
                        vv[0:PS, 0, :w],
                        vv[0:PS, 1, :w],
                        op=Alu.subtract,
                    )
                    if u < N_U - 1:
                        sub.then_inc(d_sem, 1)
                    else:
                        # final unit: |d| + accum on DVE, off the ACT handoff
                        v.scalar_tensor_tensor(
                            ma[0:PS, 0, :w],
                            dd[u % 3][0:PS, :w],
                            -1.0,
                            dd[u % 3][0:PS, :w],
                            op0=Alu.mult,
                            op1=Alu.max,
                            accum_out=acc[0:PS, u : u + 1],
                        ).then_inc(act_sem, 1)
                else:
                    v.wait_ge(in_p, 16 * 12)
                    v.wait_ge(in_t, 16 * 12)
                    v.tensor_tensor(
                        maf[:, :, :],
                        es[:, :, 0, :],
                        es[:, :, 1, :],
                        op=Alu.max,
                    )
                    v.tensor_tensor(
                        mbf[:, :, :], maf[:, :, :], es[:, :, 2, :], op=Alu.max
                    )
                    v.tensor_scalar(
                        vvf[:, :, :],
                        mbf[:, :, :],
                        0.0,
                        1.0,
                        op0=Alu.max,
                        op1=Alu.min,
                    )
                    v.tensor_tensor(
                        ddf[:, :], vvf[:, 0, :], vvf[:, 1, :], op=Alu.subtract
                    ).then_inc(d_sem, 1)

        @block.scalar
        def _(s):
            s.wait_ge(g_sem[0], 30)
            for dst, src in es_slices(1):
                s.dma_start(out=dst, in_=src).then_inc(in_t, 16)
            for u, (kind, gi) in enumerate(units[:-1]):
                s.wait_ge(d_sem, u + 1)
                if kind == "es":
                    s.activation(
                        absb[:, :ES_W],
                        ddf[:, :],
                        Act.Abs,
                        accum_out=acc[:, u : u + 1],
                    ).then_inc(act_sem, 1)
                else:
                    w = GROUPS[gi][2]
                    s.activation(
                        absb[0:PS, :w],
                        dd[u % 3][0:PS, :w],
                        Act.Abs,
                        accum_out=acc[0:PS, u : u + 1],
                    ).then_inc(act_sem, 1)

        nc.all_engine_barrier = lambda *a, **k: None

    del nc.all_engine_barrier  # restore class method
    return nc


_program = None


def _get_program():
    global _program
    if _program is None:
        _program = _build_program()
    return _program


def _finish(partials_list):
    """partials_list: per-core [P, N_U] f32, each col = sum |Vp-Vt| of a unit.
    sw columns are valid on partitions 0:PS only (120:128 never written);
    the es column is valid on all 128."""
    total = np.float64(0.0)
    for p in partials_list:
        p = p.astype(np.float64)
        total += p[:, ES_POS].sum()
        total += p[0:PS, :ES_POS].sum() + p[0:PS, ES_POS + 1 :].sum()
    return np.array(total / N_PIX, dtype=np.float32)


def kernel(pred: np.ndarray, target: np.ndarray) -> np.ndarray:
    from concourse.bass_utils import run_bass_kernel_spmd

    nc = _get_program()
    pred = np.ascontiguousarray(pred, dtype=np.float32).reshape(
        N_CORES, N_IMG, C, P, F
    )
    target = np.ascontiguousarray(target, dtype=np.float32).reshape(
        N_CORES, N_IMG, C, P, F
    )
    in_maps = [{"pred": pred[i], "target": target[i]} for i in range(N_CORES)]
    res = run_bass_kernel_spmd(nc, in_maps, list(range(N_CORES)))
    return _finish([r["partials"] for r in res.results])
